# revision 1
# baseline (speedup 1.0000x reference)
"""Trainium2 Bass kernel for nn_Net_91268055040039 (dense_mlp).

Computes out[b] = sum_{t,p} x[b,t,p] * |W[t,p]| * fc1_w[0, t*P+p] + fc1_b
  x: [32, 400, 10000] f32, W: [400, 10000] f32, fc1_w: [1, 4000000] f32.

Strategy: shard the reduction dim T=400 into 8 slices of 50 rows (64MB of x +
4MB of params per core, vs 64+32MB for batch sharding). Per core the 500000
reduction elements per batch are padded to 128*3907 and laid out
partition-major ON THE HOST, so each SBUF partition's data for consecutive
batches is contiguous in HBM. DMA then moves 8MB chunks with 62.5KB
contiguous per-partition runs (~397 GB/s measured on this setup, vs 181 GB/s
for 16KB runs - descriptor overhead dominates short runs).

Per core:
  v = |W_shard| * fc1_shard              (ACT abs + DVE mult, in-place)
  for b in 32: acc[:, b] = reduce_add(x_tile_b * v)   (one fused DVE
        scalar_tensor_tensor with accum_out per batch; tensor_tensor_reduce
        crashes this HW/runtime build)
  psum[1, 32] = ones[128,1].T @ acc[128,32]           (PE partition reduction)
Host sums the 8 per-core partials and adds fc1_b.
"""

import numpy as np

import concourse.bass as bass
import concourse.bacc as bacc
import concourse.mybir as mybir
from concourse.tile import TileContext
from concourse.bass_utils import run_bass_kernel_spmd

B, T, P = 32, 400, 10000
NCORES = 8
TS = T // NCORES          # 50 T-rows per core
K = TS * P                # 500000 reduction elements per core per batch
PART = 128
FREE = 3907               # ceil(K / PART); 128*3907 = 500096 (96 zero pad)
KPAD = PART * FREE
CHUNK = 4                 # batches per DMA: 4 * 3907 * 4B = 62.5KB per row
NCHUNKS = B // CHUNK
F32 = mybir.dt.float32

# Set by the test harness to capture an NTFF profile; harmless when False.
TRACE = False
LAST_RESULT = None


def build_program() -> bass.Bass:
    # Bacc (not raw Bass): its compile() splits multi-sem waits into separate
    # instructions - this neuronxcc build allows only 1 sync-wait per inst.
    nc = bacc.Bacc()
    xs = nc.declare_dram_parameter("xs", [PART, B * FREE], F32, isOutput=False)
    # wf[:, :FREE] = W shard, wf[:, FREE:] = fc1 shard (one DMA for both).
    wf = nc.declare_dram_parameter("wf", [PART, 2 * FREE], F32, isOutput=False)
    out = nc.declare_dram_parameter("out", [1, B], F32, isOutput=True)

    with TileContext(nc) as tc:
        with (
            tc.tile_pool(name="const", bufs=1) as cpool,
            tc.tile_pool(name="xp", bufs=2) as xpool,
            tc.tile_pool(name="psum", bufs=1, space="PSUM") as ppool,
        ):
            # Params on the sync/HWDGE ring so the gpsimd/SWDGE ring starts
            # streaming x immediately.
            wft = cpool.tile([PART, 2 * FREE], F32)
            nc.sync.dma_start(out=wft, in_=wf[:, :])
            # v = |W| * fc1, computed in place over the W half of wft.
            v = wft[:, :FREE]
            nc.scalar.activation(
                out=v, in_=v, func=mybir.ActivationFunctionType.Abs
            )
            nc.vector.tensor_tensor(
                out=v, in0=v, in1=wft[:, FREE:], op=mybir.AluOpType.mult
            )

            ones = cpool.tile([PART, 1], F32)
            nc.vector.memset(ones, 1.0)
            acc = cpool.tile([PART, B], F32)
            scratch = cpool.tile([PART, FREE], F32)

            for g in range(NCHUNKS):
                xt = xpool.tile([PART, CHUNK * FREE], F32, tag="xt")
                nc.gpsimd.dma_start(
                    out=xt, in_=xs[:, g * CHUNK * FREE : (g + 1) * CHUNK * FREE]
                )
                for c in range(CHUNK):
                    b = g * CHUNK + c
                    # Fused multiply + free-dim reduce in one DVE pass:
                    # scratch = (x_b bypass 0) mult v; acc[:, b] = sum(scratch)
                    nc.vector.scalar_tensor_tensor(
                        out=scratch,
                        in0=xt[:, c * FREE : (c + 1) * FREE],
                        scalar=0.0,
                        in1=v,
                        op0=mybir.AluOpType.bypass,
                        op1=mybir.AluOpType.mult,
                        accum_out=acc[:, b : b + 1],
                    )

            ps = ppool.tile([1, B], F32)
            nc.tensor.matmul(out=ps, lhsT=ones, rhs=acc, start=True, stop=True)
            res = cpool.tile([1, B], F32)
            nc.scalar.copy(res, ps)
            nc.sync.dma_start(out=out[:, :], in_=res)
    nc.finalize()
    return nc


def _to_partition_major(flat: np.ndarray) -> np.ndarray:
    """[N, K] row-major -> [PART, N*FREE] where each partition's rows for
    consecutive N are adjacent (N along the middle axis)."""
    n = flat.shape[0]
    padded = np.zeros((n, KPAD), dtype=np.float32)
    padded[:, :K] = flat
    # [n, PART, FREE] -> [PART, n, FREE] -> [PART, n*FREE]
    return np.ascontiguousarray(
        padded.reshape(n, PART, FREE).transpose(1, 0, 2)
    ).reshape(PART, n * FREE)


def make_in_maps(x: np.ndarray, W: np.ndarray, fc1_w: np.ndarray):
    x = np.asarray(x, dtype=np.float32)
    W = np.asarray(W, dtype=np.float32)
    fc1_w = np.asarray(fc1_w, dtype=np.float32)
    fc1_flat = fc1_w.reshape(T, P)
    in_maps = []
    for c in range(NCORES):
        t0 = c * TS
        xs = _to_partition_major(x[:, t0 : t0 + TS, :].reshape(B, K))
        ws = _to_partition_major(W[t0 : t0 + TS, :].reshape(1, K))
        fs = _to_partition_major(fc1_flat[t0 : t0 + TS, :].reshape(1, K))
        in_maps.append({"xs": xs, "wf": np.concatenate([ws, fs], axis=1)})
    return in_maps


def kernel(x, W, fc1_w, fc1_b):
    global LAST_RESULT
    nc = build_program()
    in_maps = make_in_maps(x, W, fc1_w)
    res = run_bass_kernel_spmd(
        nc, in_maps, core_ids=list(range(NCORES)), trace=TRACE
    )
    LAST_RESULT = res
    partial = np.zeros(B, dtype=np.float64)
    for r in res.results:
        partial += r["out"][0].astype(np.float64)
    out = partial.astype(np.float32) + np.float32(np.asarray(fc1_b).reshape(-1)[0])
    return out.reshape(B, 1).astype(np.float32)



# revision 2
# speedup vs baseline: 1.2366x; 1.2366x over previous
"""Trainium2 Bass kernel for nn_Net_91268055040039 (dense_mlp).

Computes out[b] = sum_{t,p} x[b,t,p] * |W[t,p]| * fc1_w[0, t*P+p] + fc1_b
  x: [32, 400, 10000] f32, W: [400, 10000] f32, fc1_w: [1, 4000000] f32.

Strategy: shard the reduction dim T=400 into 8 slices of 50 rows. The kernel
is HBM-bandwidth bound (512MB of x), so x is streamed as FP16 (half the
bytes; rel err ~2e-3 vs the 2e-2 gate — inputs are N(0,1) so fp16's 11-bit
mantissa loses ~6e-4 per element and errors average out over the 4M-term
sum). v = |W|*fc1 is precomputed on the host and also shipped fp16.

Per core the 500000 reduction elements per batch are padded to 128*3908
(FREE even so per-batch slices stay 4B-aligned - required for DVE 2x packed
mode) and laid out partition-major ON THE HOST, so each SBUF partition's
data for consecutive batches is contiguous in HBM: 62.5KB runs per DMA at
CHUNK=8 batches (descriptor overhead dominates short runs).

Per core:
  for b in 32: acc[:, b] = reduce_add(x_b * v)   (fused DVE
        scalar_tensor_tensor fp16 with f32 accum_out per batch)
  psum[1, 32] = ones[128,1].T @ acc[128,32]      (PE partition reduction)
Host sums the 8 per-core partials and adds fc1_b.
Chunk schedule tapers (8,8,8,4,2,1,1) so the last DVE op covers one batch
and the compute tail past the final DMA is minimal.
"""

import numpy as np

import concourse.bass as bass
import concourse.bacc as bacc
import concourse.mybir as mybir
from concourse.tile import TileContext
from concourse.bass_utils import run_bass_kernel_spmd

B, T, P = 32, 400, 10000
NCORES = 8
TS = T // NCORES          # 50 T-rows per core
K = TS * P                # 500000 reduction elements per core per batch
PART = 128
FREE = 3908               # ceil(K/PART) rounded up to even; 128*3908 = 500224
KPAD = PART * FREE
CHUNKS = (8, 8, 8, 4, 2, 1, 1)   # batches per DMA (sum = 32)
CHUNK_MAX = max(CHUNKS)
F32 = mybir.dt.float32
F16 = mybir.dt.float16

# Set by the test harness to capture an NTFF profile; harmless when False.
TRACE = False
LAST_RESULT = None


def build_program() -> bass.Bass:
    # Bacc (not raw Bass): its compile() splits multi-sem waits into separate
    # instructions - this neuronxcc build allows only 1 sync-wait per inst.
    nc = bacc.Bacc()
    xs = nc.declare_dram_parameter("xs", [PART, B * FREE], F16, isOutput=False)
    vd = nc.declare_dram_parameter("vd", [PART, FREE], F16, isOutput=False)
    out = nc.declare_dram_parameter("out", [1, B], F32, isOutput=True)

    with TileContext(nc) as tc:
        with (
            tc.tile_pool(name="const", bufs=1) as cpool,
            tc.tile_pool(name="xp", bufs=2) as xpool,
            tc.tile_pool(name="psum", bufs=1, space="PSUM") as ppool,
        ):
            # v on the sync/HWDGE ring so the gpsimd/SWDGE ring starts
            # streaming x immediately.
            v = cpool.tile([PART, FREE], F16)
            nc.sync.dma_start(out=v, in_=vd[:, :])

            ones = cpool.tile([PART, 1], F32)
            nc.vector.memset(ones, 1.0)
            acc = cpool.tile([PART, B], F32)
            scratch = cpool.tile([PART, FREE], F16)

            b = 0
            off = 0
            for cn in CHUNKS:
                xt = xpool.tile([PART, CHUNK_MAX * FREE], F16, tag="xt")
                nc.gpsimd.dma_start(
                    out=xt[:, : cn * FREE], in_=xs[:, off : off + cn * FREE]
                )
                off += cn * FREE
                for c in range(cn):
                    # Fused multiply + free-dim reduce in one DVE pass:
                    # scratch = (x_b bypass 0) mult v; acc[:, b] = sum(scratch)
                    nc.vector.scalar_tensor_tensor(
                        out=scratch,
                        in0=xt[:, c * FREE : (c + 1) * FREE],
                        scalar=0.0,
                        in1=v,
                        op0=mybir.AluOpType.bypass,
                        op1=mybir.AluOpType.mult,
                        accum_out=acc[:, b : b + 1],
                    )
                    b += 1

            ps = ppool.tile([1, B], F32)
            nc.tensor.matmul(out=ps, lhsT=ones, rhs=acc, start=True, stop=True)
            res = cpool.tile([1, B], F32)
            nc.scalar.copy(res, ps)
            nc.sync.dma_start(out=out[:, :], in_=res)
    nc.finalize()
    return nc


def _to_partition_major(flat: np.ndarray) -> np.ndarray:
    """[N, K] -> fp16 [PART, N*FREE] where each partition's rows for
    consecutive N are adjacent (N along the middle axis)."""
    n = flat.shape[0]
    padded = np.zeros((n, KPAD), dtype=np.float16)
    padded[:, :K] = flat
    # [n, PART, FREE] -> [PART, n, FREE] -> [PART, n*FREE]
    return np.ascontiguousarray(
        padded.reshape(n, PART, FREE).transpose(1, 0, 2)
    ).reshape(PART, n * FREE)


def make_in_maps(x: np.ndarray, W: np.ndarray, fc1_w: np.ndarray):
    x = np.asarray(x)
    v_full = np.abs(np.asarray(W, dtype=np.float32)) * np.asarray(
        fc1_w, dtype=np.float32
    ).reshape(T, P)
    in_maps = []
    for c in range(NCORES):
        t0 = c * TS
        xs = _to_partition_major(x[:, t0 : t0 + TS, :].reshape(B, K))
        vs = _to_partition_major(v_full[t0 : t0 + TS].reshape(1, K))
        in_maps.append({"xs": xs, "vd": vs})
    return in_maps


def kernel(x, W, fc1_w, fc1_b):
    global LAST_RESULT
    nc = build_program()
    in_maps = make_in_maps(x, W, fc1_w)
    res = run_bass_kernel_spmd(
        nc, in_maps, core_ids=list(range(NCORES)), trace=TRACE
    )
    LAST_RESULT = res
    partial = np.zeros(B, dtype=np.float64)
    for r in res.results:
        partial += r["out"][0].astype(np.float64)
    out = partial.astype(np.float32) + np.float32(np.asarray(fc1_b).reshape(-1)[0])
    return out.reshape(B, 1).astype(np.float32)


# revision 7
# speedup vs baseline: 1.9887x; 1.6082x over previous
"""Trainium2 Bass kernel for nn_Net_91268055040039 (dense_mlp).

Computes out[b] = sum_{t,p} x[b,t,p] * |W[t,p]| * fc1_w[0, t*P+p] + fc1_b
  x: [32, 400, 10000] f32, W: [400, 10000] f32, fc1_w: [1, 4000000] f32.

Strategy: shard the reduction dim T=400 into 8 slices of 50 rows. The kernel
is HBM-bandwidth bound (512MB of x), so x is streamed as FP16 (half the
bytes; rel err ~2e-3 vs the 2e-2 gate - inputs are N(0,1), errors average
out over the 4M-term sum). v = |W|*fc1 is precomputed on the host, fp16.

The multiply+reduce runs on the TENSOR engine (DVE scalar_tensor_tensor has
no 2x fp16 uop - measured 4.2us per 3908-elem op = 135us total, the
bottleneck of the previous version). Layout is k-major: partition p holds
k = n*128 + p, so PE contracts 128 k-values per matmul. To beat the
60-cycle-per-matmul floor, G=8 k-groups share one matmul via the diagonal
trick: lhsT = v[:, n:n+8] (8 cols), rhs = x[:, (n,b) block] [128, 8*32],
accumulating psum[8, 256] over all 489 groups; only the g==g' diagonal
[g, g*32:(g+1)*32] is wanted (the off-diag products are discarded at the
end). PE cost ~= 256 rows * 0.42ns * 489 = 53us < ~85us DMA floor.

DMA: per-partition contiguous run = chunk_n*32*2 bytes, so chunks of ~976
n-groups give 62.5KB runs (descriptor overhead dominates short runs). The
chunk schedule tapers so the compute tail past the final DMA is tiny.

End per core: 8 scalar copies extract the psum diagonal -> [8, 32] sbuf,
ones[8,1].T @ that -> [1, 32]. Host sums the 8 per-core partials + fc1_b.
"""

import numpy as np

import concourse.bass as bass
import concourse.bacc as bacc
import concourse.mybir as mybir
from concourse.tile import TileContext
from concourse.bass_utils import run_bass_kernel_spmd

B, T, P = 32, 400, 10000
NCORES = 8
TS = T // NCORES          # 50 T-rows per core
K = TS * P                # 500000 reduction elements per core per batch
PART = 128
G = 8                     # k-groups (of 128) packed per matmul
NJG = 3912                # ceil(K/128)=3907 rounded up to a multiple of G
KPAD = NJG * PART         # 500736 (736 zero pad)
CHUNKS = (976, 976, 976, 792, 128, 48, 16)   # n-groups per DMA (sum = NJG)
CHUNK_MAX = max(CHUNKS)
F32 = mybir.dt.float32
F16 = mybir.dt.float16

# Set by the test harness to capture an NTFF profile; harmless when False.
TRACE = False
LAST_RESULT = None


def build_program() -> bass.Bass:
    # Bacc (not raw Bass): its compile() splits multi-sem waits into separate
    # instructions - this neuronxcc build allows only 1 sync-wait per inst.
    nc = bacc.Bacc()
    xs = nc.declare_dram_parameter("xs", [PART, NJG * B], F16, isOutput=False)
    vd = nc.declare_dram_parameter("vd", [PART, NJG], F16, isOutput=False)
    out = nc.declare_dram_parameter("out", [G, G * B], F32, isOutput=True)

    with TileContext(nc) as tc:
        with (
            tc.tile_pool(name="const", bufs=1) as cpool,
            tc.tile_pool(name="xp", bufs=2) as xpool,
            tc.tile_pool(name="psum", bufs=1, space="PSUM") as ppool,
        ):
            # v on the gpsimd/SWDGE ring; x streams on the sync/HWDGE ring
            # (SWDGE descriptor rings contend with DMA engine 15 - measured
            # 96.7us vs 81us busy on the x stream when it ran on gpsimd).
            vt = cpool.tile([PART, NJG], F16)
            nc.gpsimd.dma_start(out=vt, in_=vd[:, :])

            ps = ppool.tile([G, G * B], F32)
            nmm = NJG // G
            mm = 0
            n0 = 0
            for cn in CHUNKS:
                xt = xpool.tile([PART, CHUNK_MAX * B], F16, tag="xt")
                nc.sync.dma_start(
                    out=xt[:, : cn * B], in_=xs[:, n0 * B : (n0 + cn) * B]
                )
                for i in range(cn // G):
                    nc.tensor.matmul(
                        out=ps,
                        lhsT=vt[:, n0 + i * G : n0 + (i + 1) * G],
                        rhs=xt[:, i * G * B : (i + 1) * G * B],
                        start=(mm == 0),
                        stop=(mm == nmm - 1),
                    )
                    mm += 1
                n0 += cn

            # Ship the whole [G, G*B] accumulator; the host extracts the
            # diagonal blocks psum[g, g*B:(g+1)*B] and sums over g (8KB out).
            res = cpool.tile([G, G * B], F32)
            nc.scalar.copy(res, ps)
            nc.sync.dma_start(out=out[:, :], in_=res)
    nc.finalize()
    return nc


def make_in_maps(x: np.ndarray, W: np.ndarray, fc1_w: np.ndarray):
    x = np.asarray(x)
    v_full = np.abs(np.asarray(W, dtype=np.float32)) * np.asarray(
        fc1_w, dtype=np.float32
    ).reshape(T, P)
    in_maps = []
    for c in range(NCORES):
        t0 = c * TS
        # x k-major: xs[p, n*B + b] = x[b, k=n*128+p]
        xpad = np.zeros((B, KPAD), dtype=np.float16)
        xpad[:, :K] = x[:, t0 : t0 + TS, :].reshape(B, K)
        xs = np.ascontiguousarray(
            xpad.reshape(B, NJG, PART).transpose(2, 1, 0)
        ).reshape(PART, NJG * B)
        vpad = np.zeros(KPAD, dtype=np.float16)
        vpad[:K] = v_full[t0 : t0 + TS].reshape(-1)
        vs = np.ascontiguousarray(vpad.reshape(NJG, PART).T)
        in_maps.append({"xs": xs, "vd": vs})
    return in_maps


def kernel(x, W, fc1_w, fc1_b):
    global LAST_RESULT
    nc = build_program()
    in_maps = make_in_maps(x, W, fc1_w)
    res = run_bass_kernel_spmd(
        nc, in_maps, core_ids=list(range(NCORES)), trace=TRACE
    )
    LAST_RESULT = res
    partial = np.zeros(B, dtype=np.float64)
    for r in res.results:
        o = r["out"].astype(np.float64)          # [G, G*B]
        for g in range(G):
            partial += o[g, g * B : (g + 1) * B]
    out = partial.astype(np.float32) + np.float32(np.asarray(fc1_b).reshape(-1)[0])
    return out.reshape(B, 1).astype(np.float32)
